# revision 2
# baseline (speedup 1.0000x reference)
"""MoE layer (E=8 experts, top-2 routing) on 8 Trainium2 NeuronCores.

Expert-parallel sharding: core e holds expert e's weights (w1/w2/b1/b2).
Tokens are dispatched to the cores of their top-2 experts, each core runs
its expert's FFN on its gathered tokens and scales rows by the combine
weight, and the scaled contributions are summed back per token (the
all-to-all "return") to form the full output.

Shapes (hardcoded per the problem spec):
  x [2, 2048, 512] f32, router_w [8, 512], w1_all [8, 2048, 512],
  b1_all [8, 2048], w2_all [8, 512, 2048], b2_all [8, 512].
"""

import sys

sys.path.insert(0, "/opt/trn_rl_repo")

import numpy as np

import concourse.bass as bass
import concourse.mybir as mybir
import concourse.tile as tile
from concourse import bacc

D_MODEL = 512
DFF = 2048
E = 8
K = 2
L = 2 * 2048  # total tokens
N_CORES = 8

FP = mybir.dt.float32

# Per-expert token capacity (padded). Expected load is L*K/E = 1024 with
# std ~30 under the near-uniform router; 1280 leaves ample slack.
CAP = 1280

_PROG_CACHE: dict = {}


def build_program(cap: int):
    """One SPMD program, run on all 8 cores; per-core data selects the expert.

    Per-core inputs:
      xgT  [512, cap]   gathered tokens for this expert, transposed (d-major)
      w1t  [512, 2048]  w1_e.T
      w2t  [2048, 512]  w2_e.T
      b1r  [16, 128, 1] b1_e
      b2r  [1, 512]     b2_e
      wgt  [cap//128, 128, 1] combine weights per gathered slot (0 for pads)
    Output:
      out  [cap, 512]   scaled expert contributions, row s = token slot s
    """
    nc = bacc.Bacc("TRN2", target_bir_lowering=False, debug=True)

    xgT = nc.dram_tensor("xgT", [D_MODEL, cap], FP, kind="ExternalInput")
    w1t = nc.dram_tensor("w1t", [D_MODEL, DFF], FP, kind="ExternalInput")
    w2t = nc.dram_tensor("w2t", [DFF, D_MODEL], FP, kind="ExternalInput")
    b1r = nc.dram_tensor("b1r", [DFF // 128, 128, 1], FP, kind="ExternalInput")
    b2r = nc.dram_tensor("b2r", [1, D_MODEL], FP, kind="ExternalInput")
    wgt = nc.dram_tensor("wgt", [cap // 128, 128, 1], FP, kind="ExternalInput")
    out = nc.dram_tensor("out", [cap, D_MODEL], FP, kind="ExternalOutput")

    KD = D_MODEL // 128  # 4 k-slices for mm1
    MD = DFF // 128  # 16 dff tiles
    n_blocks = (cap + 511) // 512  # token blocks of <=512 for mm1 N-dim

    with tile.TileContext(nc) as tc:
        with (
            tc.tile_pool(name="weights", bufs=1) as wpool,
            tc.tile_pool(name="acts", bufs=2) as apool,
            tc.tile_pool(name="h", bufs=2) as hpool,
            tc.tile_pool(name="psum", bufs=4, space="PSUM") as ppool,
            tc.tile_pool(name="outp", bufs=3) as opool,
            tc.tile_pool(name="consts", bufs=1) as cpool,
        ):
            # --- load weights / constants into SBUF ---
            w1t_sb = []
            for k in range(KD):
                t = wpool.tile([128, DFF], FP, tag=f"w1t{k}")
                nc.sync.dma_start(out=t[:], in_=w1t[k * 128 : (k + 1) * 128, :])
                w1t_sb.append(t)
            w2t_sb = []
            for m in range(MD):
                t = wpool.tile([128, D_MODEL], FP, tag=f"w2t{m}")
                nc.sync.dma_start(out=t[:], in_=w2t[m * 128 : (m + 1) * 128, :])
                w2t_sb.append(t)
            xgT_sb = []
            for k in range(KD):
                t = wpool.tile([128, cap], FP, tag=f"xgT{k}")
                nc.sync.dma_start(out=t[:], in_=xgT[k * 128 : (k + 1) * 128, :])
                xgT_sb.append(t)
            b1_sb = wpool.tile([128, MD], FP, tag="b1")
            # b1r is [16,128,1]; lay tiles side by side: column m = tile m
            nc.sync.dma_start(
                out=b1_sb[:], in_=b1r.rearrange("m p o -> p (m o)")
            )
            b2_sb = cpool.tile([1, D_MODEL], FP, tag="b2")
            nc.sync.dma_start(out=b2_sb[:], in_=b2r[:, :])
            wgt_sb = wpool.tile([128, cap // 128], FP, tag="wgt")
            nc.sync.dma_start(out=wgt_sb[:], in_=wgt.rearrange("u p o -> p (u o)"))
            ones_sb = cpool.tile([1, 128], FP, tag="ones")
            nc.vector.memset(ones_sb[:], 1.0)

            # --- main loop over token blocks ---
            for blk in range(n_blocks):
                ncols = min(512, cap - blk * 512)
                # mm1: h_T[dff, tokens-in-block] = relu(w1t.T @ xgT + b1)
                h_sb = []
                for m in range(MD):
                    ps = ppool.tile([128, ncols], FP, tag="ps1")
                    for k in range(KD):
                        nc.tensor.matmul(
                            ps[:],
                            w1t_sb[k][:, m * 128 : (m + 1) * 128],
                            xgT_sb[k][:, blk * 512 : blk * 512 + ncols],
                            start=(k == 0),
                            stop=(k == KD - 1),
                        )
                    h = hpool.tile([128, ncols], FP, tag=f"h{m}")
                    nc.scalar.activation(
                        h[:],
                        ps[:],
                        mybir.ActivationFunctionType.Relu,
                        bias=b1_sb[:, m : m + 1],
                    )
                    h_sb.append(h)
                # mm2: out[tok, d] = (h_T.T @ w2t) + b2, then scale rows
                for t in range(ncols // 128):
                    ps2 = ppool.tile([128, D_MODEL], FP, tag="ps2")
                    for m in range(MD):
                        nc.tensor.matmul(
                            ps2[:],
                            h_sb[m][:, t * 128 : (t + 1) * 128],
                            w2t_sb[m][:],
                            start=(m == 0),
                            stop=False,
                        )
                    # rank-1 bias: ones[1,128].T @ b2[1,512]
                    nc.tensor.matmul(
                        ps2[:], ones_sb[:], b2_sb[:], start=False, stop=True
                    )
                    o = opool.tile([128, D_MODEL], FP, tag="o")
                    u = blk * 4 + t  # token subtile index
                    nc.scalar.mul(o[:], ps2[:], mul=wgt_sb[:, u : u + 1])
                    nc.sync.dma_start(
                        out=out[u * 128 : (u + 1) * 128, :], in_=o[:]
                    )
    nc.compile()
    return nc


def _route(x_flat: np.ndarray, router_w: np.ndarray):
    """Host-side replica of the reference router: top-2 + renormalized weights."""
    logits = x_flat @ router_w.T  # [L, E]
    m = logits.max(axis=-1, keepdims=True)
    p = np.exp(logits - m)
    p /= p.sum(axis=-1, keepdims=True)
    order = np.argsort(-p, axis=-1)[:, :K]  # [L, K]
    pv = np.take_along_axis(p, order, axis=-1)
    pv = pv / (pv.sum(axis=-1, keepdims=True) + 1e-9)
    return order, pv


def kernel(x, router_w, w1_all, b1_all, w2_all, b2_all):
    from concourse.bass_utils import run_bass_kernel_spmd

    x = np.asarray(x, dtype=np.float32)
    router_w = np.asarray(router_w, dtype=np.float32)
    w1_all = np.asarray(w1_all, dtype=np.float32)
    b1_all = np.asarray(b1_all, dtype=np.float32)
    w2_all = np.asarray(w2_all, dtype=np.float32)
    b2_all = np.asarray(b2_all, dtype=np.float32)

    Bb, Nn, C = x.shape
    x_flat = x.reshape(-1, C)

    # Router (replicated, tiny) + expert-parallel dispatch lists.
    order, pv = _route(x_flat, router_w)
    idx_lists = []
    wgt_lists = []
    for e in range(E):
        sel = np.nonzero(order == e)
        toks = sel[0]
        ws = pv[sel]
        assert len(toks) <= CAP, f"expert {e} overflow: {len(toks)} > {CAP}"
        idx_lists.append(toks)
        wgt_lists.append(ws)

    if "prog" not in _PROG_CACHE:
        _PROG_CACHE["prog"] = build_program(CAP)
    nc = _PROG_CACHE["prog"]

    in_maps = []
    for e in range(E):
        toks, ws = idx_lists[e], wgt_lists[e]
        n_e = len(toks)
        xg = np.zeros((CAP, C), np.float32)
        xg[:n_e] = x_flat[toks]
        wg = np.zeros((CAP,), np.float32)
        wg[:n_e] = ws
        in_maps.append(
            {
                "xgT": np.ascontiguousarray(xg.T),
                "w1t": np.ascontiguousarray(w1_all[e].T),
                "w2t": np.ascontiguousarray(w2_all[e].T),
                "b1r": np.ascontiguousarray(b1_all[e].reshape(DFF // 128, 128, 1)),
                "b2r": np.ascontiguousarray(b2_all[e].reshape(1, C)),
                "wgt": np.ascontiguousarray(wg.reshape(CAP // 128, 128, 1)),
            }
        )

    res = run_bass_kernel_spmd(nc, in_maps, core_ids=list(range(N_CORES)))

    # Unshard: weighted all-to-all return == scatter-add contributions per token.
    final = np.zeros((Bb * Nn, C), np.float32)
    for e in range(E):
        toks = idx_lists[e]
        final[toks] += res.results[e]["out"][: len(toks)]
    return final.reshape(Bb, Nn, C)


def _build_in_maps(x, router_w, w1_all, b1_all, w2_all, b2_all):
    """Shared staging used by kernel() and the timing harness."""
    x_flat = np.asarray(x, np.float32).reshape(-1, D_MODEL)
    order, pv = _route(x_flat, np.asarray(router_w, np.float32))
    in_maps = []
    idx_lists = []
    for e in range(E):
        sel = np.nonzero(order == e)
        toks, ws = sel[0], pv[sel]
        idx_lists.append(toks)
        n_e = len(toks)
        xg = np.zeros((CAP, D_MODEL), np.float32)
        xg[:n_e] = x_flat[toks]
        wg = np.zeros((CAP,), np.float32)
        wg[:n_e] = ws
        in_maps.append(
            {
                "xgT": np.ascontiguousarray(xg.T),
                "w1t": np.ascontiguousarray(np.asarray(w1_all)[e].T.astype(np.float32)),
                "w2t": np.ascontiguousarray(np.asarray(w2_all)[e].T.astype(np.float32)),
                "b1r": np.asarray(b1_all, np.float32)[e].reshape(DFF // 128, 128, 1),
                "b2r": np.asarray(b2_all, np.float32)[e].reshape(1, D_MODEL),
                "wgt": wg.reshape(CAP // 128, 128, 1),
            }
        )
    return in_maps, idx_lists


def time_kernel(x, router_w, w1_all, b1_all, w2_all, b2_all, iters: int = 50):
    """Wall-clock the NEFF execution: jit once, device-put inputs, run a
    pipelined loop. Returns estimated ns per execution (all 8 cores)."""
    import time as _time

    import jax
    from jax.experimental.shard_map import shard_map
    from jax.sharding import Mesh, NamedSharding, PartitionSpec

    from concourse import bass2jax

    if "prog" not in _PROG_CACHE:
        _PROG_CACHE["prog"] = build_program(CAP)
    nc = _PROG_CACHE["prog"]
    in_maps, _ = _build_in_maps(x, router_w, w1_all, b1_all, w2_all, b2_all)

    bass2jax.install_neuronx_cc_hook()

    import concourse.mybir as _mb

    partition_name = nc.partition_id_tensor.name if nc.partition_id_tensor else None
    in_names, out_names, out_avals, zero_outs = [], [], [], []
    for alloc in nc.m.functions[0].allocations:
        if not isinstance(alloc, _mb.MemoryLocationSet):
            continue
        name = alloc.memorylocations[0].name
        if alloc.kind == "ExternalInput":
            if name != partition_name:
                in_names.append(name)
        elif alloc.kind == "ExternalOutput":
            shape = tuple(alloc.tensor_shape)
            dtype = _mb.dt.np(alloc.dtype)
            out_names.append(name)
            out_avals.append(jax.core.ShapedArray(shape, dtype))
            zero_outs.append(np.zeros(shape, dtype))
    n_params = len(in_names)
    all_in_names = list(in_names) + list(out_names)
    if partition_name is not None:
        all_in_names.append(partition_name)
    if nc.dbg_addr is not None:
        extra_dbg = {nc.dbg_addr.name: np.zeros((1, 2), np.uint32)}
        in_maps = [{**m, **extra_dbg} for m in in_maps]

    def _body(*args):
        operands = list(args)
        if partition_name is not None:
            operands.append(bass2jax.partition_id_tensor())
        outs = bass2jax._bass_exec_p.bind(
            *operands,
            out_avals=tuple(out_avals),
            in_names=tuple(all_in_names),
            out_names=tuple(out_names),
            lowering_input_output_aliases=(),
            sim_require_finite=True,
            sim_require_nnan=True,
            nc=nc,
        )
        return tuple(outs)

    devices = jax.devices()[:N_CORES]
    mesh = Mesh(np.asarray(devices), ("core",))
    spec = PartitionSpec("core")
    in_specs = (spec,) * (n_params + len(out_names))
    out_specs = (spec,) * len(out_names)
    fn = jax.jit(
        shard_map(_body, mesh=mesh, in_specs=in_specs, out_specs=out_specs,
                  check_rep=False),
        keep_unused=True,
    )
    sharding = NamedSharding(mesh, spec)
    concat_in = [
        jax.device_put(
            np.concatenate([np.asarray(in_maps[c][n]) for c in range(N_CORES)], axis=0),
            sharding,
        )
        for n in in_names[:n_params]
    ]
    concat_zeros = [
        jax.device_put(
            np.zeros((N_CORES * z.shape[0], *z.shape[1:]), z.dtype), sharding
        )
        for z in zero_outs
    ]
    # warmup + compile
    outs = fn(*concat_in, *concat_zeros)
    jax.block_until_ready(outs)

    t0 = _time.perf_counter()
    for _ in range(iters):
        outs = fn(*concat_in, *concat_zeros)
    jax.block_until_ready(outs)
    dt = _time.perf_counter() - t0
    return dt / iters * 1e9


# revision 9
# speedup vs baseline: 23.2968x; 23.2968x over previous
"""MoE layer (E=8 experts, top-2 routing) on 8 Trainium2 NeuronCores.

Expert-parallel sharding: core e holds expert e's weights (w1/w2/b1/b2).
Tokens are dispatched to the cores of their top-2 experts, each core runs
its expert's FFN on its gathered tokens and scales rows by the combine
weight, and the scaled contributions are summed back per token (the
all-to-all "return") to form the full output.

Shapes (hardcoded per the problem spec):
  x [2, 2048, 512] f32, router_w [8, 512], w1_all [8, 2048, 512],
  b1_all [8, 2048], w2_all [8, 512, 2048], b2_all [8, 512].
"""

import sys

sys.path.insert(0, "/opt/trn_rl_repo")

import numpy as np

import concourse.bass as bass
import concourse.mybir as mybir
import concourse.tile as tile
from concourse import bacc

D_MODEL = 512
DFF = 2048
E = 8
K = 2
L = 2 * 2048  # total tokens
N_CORES = 8

FP = mybir.dt.float32

# Per-expert token capacity (padded). Expected load is L*K/E = 1024 with
# std ~30 under the near-uniform router; seed-0 max count is 1092. The
# program is built for the actual max count rounded up, so this is only
# the floor.
CAP = 1152

# float32r: single-pass fp32 matmul (full rate for moving dim >= 256),
# vs plain fp32 which runs as two half-speed passes.
MMDT = mybir.dt.float32r

_PROG_CACHE: dict = {}


def build_program(cap: int):
    """One SPMD program, run on all 8 cores; per-core data selects the expert.

    Per-core inputs:
      xgT  [512, cap]   gathered tokens for this expert, transposed (d-major)
      w1t  [512, 2048]  w1_e.T
      w2t  [2048, 512]  w2_e.T
      b1r  [16, 128, 1] b1_e
      b2r  [1, 512]     b2_e
      wgt  [cap//128, 128, 1] combine weights per gathered slot (0 for pads)
    Output:
      out  [cap, 512]   scaled expert contributions, row s = token slot s
    """
    nc = bacc.Bacc("TRN2", target_bir_lowering=False, debug=True)

    xgT = nc.dram_tensor("xgT", [D_MODEL, cap], MMDT, kind="ExternalInput")
    w1t = nc.dram_tensor("w1t", [D_MODEL, DFF], MMDT, kind="ExternalInput")
    w2t = nc.dram_tensor("w2t", [DFF, D_MODEL], MMDT, kind="ExternalInput")
    b1r = nc.dram_tensor("b1r", [DFF // 128, 128, 1], FP, kind="ExternalInput")
    b2r = nc.dram_tensor("b2r", [1, D_MODEL], MMDT, kind="ExternalInput")
    wgt = nc.dram_tensor("wgt", [cap // 128, 128, 1], FP, kind="ExternalInput")
    onesr = nc.dram_tensor("onesr", [1, 128], MMDT, kind="ExternalInput")
    out = nc.dram_tensor("out", [cap, D_MODEL], FP, kind="ExternalOutput")

    KD = D_MODEL // 128  # 4 k-slices for mm1
    MD = DFF // 128  # 16 dff tiles
    n_blocks = (cap + 511) // 512  # token blocks of <=512 for mm1 N-dim

    with tile.TileContext(nc) as tc:
        with (
            tc.tile_pool(name="weights", bufs=1) as wpool,
            tc.tile_pool(name="acts", bufs=2) as apool,
            tc.tile_pool(name="h", bufs=2) as hpool,
            tc.tile_pool(name="psum", bufs=4, space="PSUM") as ppool,
            tc.tile_pool(name="outp", bufs=3) as opool,
            tc.tile_pool(name="consts", bufs=1) as cpool,
        ):
            # --- load weights / constants into SBUF ---
            w1t_sb = []
            for k in range(KD):
                t = wpool.tile([128, DFF], MMDT, tag=f"w1t{k}")
                nc.sync.dma_start(out=t[:], in_=w1t[k * 128 : (k + 1) * 128, :])
                w1t_sb.append(t)
            w2t_sb = []
            for m in range(MD):
                t = wpool.tile([128, D_MODEL], MMDT, tag=f"w2t{m}")
                nc.sync.dma_start(out=t[:], in_=w2t[m * 128 : (m + 1) * 128, :])
                w2t_sb.append(t)
            xgT_sb = []
            for k in range(KD):
                t = wpool.tile([128, cap], MMDT, tag=f"xgT{k}")
                nc.sync.dma_start(out=t[:], in_=xgT[k * 128 : (k + 1) * 128, :])
                xgT_sb.append(t)
            b1_sb = wpool.tile([128, MD], FP, tag="b1")
            # b1r is [16,128,1]; lay tiles side by side: column m = tile m
            nc.sync.dma_start(
                out=b1_sb[:], in_=b1r.rearrange("m p o -> p (m o)")
            )
            b2_sb = cpool.tile([1, D_MODEL], MMDT, tag="b2")
            nc.sync.dma_start(out=b2_sb[:], in_=b2r[:, :])
            wgt_sb = wpool.tile([128, cap // 128], FP, tag="wgt")
            nc.sync.dma_start(out=wgt_sb[:], in_=wgt.rearrange("u p o -> p (u o)"))
            ones_sb = cpool.tile([1, 128], MMDT, tag="ones")
            nc.sync.dma_start(out=ones_sb[:], in_=onesr[:, :])

            # --- main loop over token blocks ---
            for blk in range(n_blocks):
                ncols = min(512, cap - blk * 512)
                # mm1: h_T[dff, tokens-in-block] = relu(w1t.T @ xgT + b1)
                h_sb = []
                for m in range(MD):
                    ps = ppool.tile([128, ncols], FP, tag="ps1")
                    for k in range(KD):
                        nc.tensor.matmul(
                            ps[:],
                            w1t_sb[k][:, m * 128 : (m + 1) * 128],
                            xgT_sb[k][:, blk * 512 : blk * 512 + ncols],
                            start=(k == 0),
                            stop=(k == KD - 1),
                        )
                    h = hpool.tile([128, ncols], MMDT, tag=f"h{m}")
                    nc.scalar.activation(
                        h[:],
                        ps[:],
                        mybir.ActivationFunctionType.Relu,
                        bias=b1_sb[:, m : m + 1],
                    )
                    h_sb.append(h)
                # mm2: out[tok, d] = (h_T.T @ w2t) + b2, then scale rows
                for t in range(ncols // 128):
                    ps2 = ppool.tile([128, D_MODEL], FP, tag="ps2")
                    for m in range(MD):
                        nc.tensor.matmul(
                            ps2[:],
                            h_sb[m][:, t * 128 : (t + 1) * 128],
                            w2t_sb[m][:],
                            start=(m == 0),
                            stop=False,
                        )
                    # rank-1 bias: ones[1,128].T @ b2[1,512]
                    nc.tensor.matmul(
                        ps2[:],
                        ones_sb[:],
                        b2_sb[:],
                        start=False,
                        stop=True,
                    )
                    o = opool.tile([128, D_MODEL], FP, tag="o")
                    u = blk * 4 + t  # token subtile index
                    nc.scalar.mul(o[:], ps2[:], mul=wgt_sb[:, u : u + 1])
                    nc.sync.dma_start(
                        out=out[u * 128 : (u + 1) * 128, :], in_=o[:]
                    )
    nc.compile()
    return nc


def _route(x_flat: np.ndarray, router_w: np.ndarray):
    """Host-side replica of the reference router: top-2 + renormalized weights."""
    logits = x_flat @ router_w.T  # [L, E]
    m = logits.max(axis=-1, keepdims=True)
    p = np.exp(logits - m)
    p /= p.sum(axis=-1, keepdims=True)
    order = np.argsort(-p, axis=-1)[:, :K]  # [L, K]
    pv = np.take_along_axis(p, order, axis=-1)
    pv = pv / (pv.sum(axis=-1, keepdims=True) + 1e-9)
    return order, pv


def _build_in_maps(x, router_w, w1_all, b1_all, w2_all, b2_all):
    """Shared staging: router + expert-parallel dispatch lists + per-core
    input maps. Returns (cap, in_maps, idx_lists)."""
    x_flat = np.asarray(x, np.float32).reshape(-1, D_MODEL)
    order, pv = _route(x_flat, np.asarray(router_w, np.float32))
    idx_lists, wgt_lists = [], []
    for e in range(E):
        sel = np.nonzero(order == e)
        idx_lists.append(sel[0])
        wgt_lists.append(pv[sel])
    max_n = max(len(t) for t in idx_lists)
    cap = max(CAP, -(-max_n // 128) * 128)
    in_maps = []
    for e in range(E):
        toks, ws = idx_lists[e], wgt_lists[e]
        n_e = len(toks)
        xg = np.zeros((cap, D_MODEL), np.float32)
        xg[:n_e] = x_flat[toks]
        wg = np.zeros((cap,), np.float32)
        wg[:n_e] = ws
        in_maps.append(
            {
                "xgT": np.ascontiguousarray(xg.T),
                "w1t": np.ascontiguousarray(np.asarray(w1_all, np.float32)[e].T),
                "w2t": np.ascontiguousarray(np.asarray(w2_all, np.float32)[e].T),
                "b1r": np.ascontiguousarray(
                    np.asarray(b1_all, np.float32)[e].reshape(DFF // 128, 128, 1)
                ),
                "b2r": np.asarray(b2_all, np.float32)[e].reshape(1, D_MODEL),
                "wgt": wg.reshape(cap // 128, 128, 1),
                "onesr": np.ones((1, 128), np.float32),
            }
        )
    return cap, in_maps, idx_lists


def _get_program(cap: int):
    if cap not in _PROG_CACHE:
        _PROG_CACHE[cap] = build_program(cap)
    return _PROG_CACHE[cap]


def kernel(x, router_w, w1_all, b1_all, w2_all, b2_all):
    from concourse.bass_utils import run_bass_kernel_spmd

    x = np.asarray(x, dtype=np.float32)
    Bb, Nn, C = x.shape

    cap, in_maps, idx_lists = _build_in_maps(
        x, router_w, w1_all, b1_all, w2_all, b2_all
    )
    nc = _get_program(cap)

    res = run_bass_kernel_spmd(nc, in_maps, core_ids=list(range(N_CORES)))

    # Unshard: weighted all-to-all return == scatter-add contributions per token.
    final = np.zeros((Bb * Nn, C), np.float32)
    for e in range(E):
        toks = idx_lists[e]
        final[toks] += res.results[e]["out"][: len(toks)]
    return final.reshape(Bb, Nn, C)


def time_kernel(x, router_w, w1_all, b1_all, w2_all, b2_all, iters: int = 50):
    """Wall-clock the NEFF execution: jit once, device-put inputs, run a
    pipelined loop. Returns estimated ns per execution (all 8 cores)."""
    import time as _time

    import jax
    from jax.experimental.shard_map import shard_map
    from jax.sharding import Mesh, NamedSharding, PartitionSpec

    from concourse import bass2jax

    cap, in_maps, _ = _build_in_maps(x, router_w, w1_all, b1_all, w2_all, b2_all)
    nc = _get_program(cap)

    bass2jax.install_neuronx_cc_hook()

    import concourse.mybir as _mb

    partition_name = nc.partition_id_tensor.name if nc.partition_id_tensor else None
    in_names, out_names, out_avals, zero_outs = [], [], [], []
    for alloc in nc.m.functions[0].allocations:
        if not isinstance(alloc, _mb.MemoryLocationSet):
            continue
        name = alloc.memorylocations[0].name
        if alloc.kind == "ExternalInput":
            if name != partition_name:
                in_names.append(name)
        elif alloc.kind == "ExternalOutput":
            shape = tuple(alloc.tensor_shape)
            dtype = _mb.dt.np(alloc.dtype)
            out_names.append(name)
            out_avals.append(jax.core.ShapedArray(shape, dtype))
            zero_outs.append(np.zeros(shape, dtype))
    n_params = len(in_names)
    all_in_names = list(in_names) + list(out_names)
    if partition_name is not None:
        all_in_names.append(partition_name)
    if nc.dbg_addr is not None:
        extra_dbg = {nc.dbg_addr.name: np.zeros((1, 2), np.uint32)}
        in_maps = [{**m, **extra_dbg} for m in in_maps]

    def _body(*args):
        operands = list(args)
        if partition_name is not None:
            operands.append(bass2jax.partition_id_tensor())
        outs = bass2jax._bass_exec_p.bind(
            *operands,
            out_avals=tuple(out_avals),
            in_names=tuple(all_in_names),
            out_names=tuple(out_names),
            lowering_input_output_aliases=(),
            sim_require_finite=True,
            sim_require_nnan=True,
            nc=nc,
        )
        return tuple(outs)

    devices = jax.devices()[:N_CORES]
    mesh = Mesh(np.asarray(devices), ("core",))
    spec = PartitionSpec("core")
    in_specs = (spec,) * (n_params + len(out_names))
    out_specs = (spec,) * len(out_names)
    fn = jax.jit(
        shard_map(_body, mesh=mesh, in_specs=in_specs, out_specs=out_specs,
                  check_rep=False),
        keep_unused=True,
    )
    sharding = NamedSharding(mesh, spec)
    concat_in = [
        jax.device_put(
            np.concatenate([np.asarray(in_maps[c][n]) for c in range(N_CORES)], axis=0),
            sharding,
        )
        for n in in_names[:n_params]
    ]
    concat_zeros = [
        jax.device_put(
            np.zeros((N_CORES * z.shape[0], *z.shape[1:]), z.dtype), sharding
        )
        for z in zero_outs
    ]
    # warmup + compile
    outs = fn(*concat_in, *concat_zeros)
    jax.block_until_ready(outs)

    t0 = _time.perf_counter()
    for _ in range(iters):
        outs = fn(*concat_in, *concat_zeros)
    jax.block_until_ready(outs)
    dt = _time.perf_counter() - t0
    return dt / iters * 1e9


# revision 10
# speedup vs baseline: 25.4513x; 1.0925x over previous
"""MoE layer (E=8 experts, top-2 routing) on 8 Trainium2 NeuronCores.

Expert-parallel sharding: core e holds expert e's weights (w1/w2/b1/b2).
Tokens are dispatched to the cores of their top-2 experts, each core runs
its expert's FFN on its gathered tokens and scales rows by the combine
weight, and the scaled contributions are summed back per token (the
all-to-all "return") to form the full output.

Shapes (hardcoded per the problem spec):
  x [2, 2048, 512] f32, router_w [8, 512], w1_all [8, 2048, 512],
  b1_all [8, 2048], w2_all [8, 512, 2048], b2_all [8, 512].
"""

import sys

sys.path.insert(0, "/opt/trn_rl_repo")

import numpy as np

import concourse.bass as bass
import concourse.mybir as mybir
import concourse.tile as tile
from concourse import bacc

D_MODEL = 512
DFF = 2048
E = 8
K = 2
L = 2 * 2048  # total tokens
N_CORES = 8

FP = mybir.dt.float32

# Per-expert token capacity (padded). Expected load is L*K/E = 1024 with
# std ~30 under the near-uniform router; seed-0 max count is 1092. The
# program is built for the actual max count rounded up, so this is only
# the floor.
CAP = 1152

# float32r: single-pass fp32 matmul (full rate for moving dim >= 256),
# vs plain fp32 which runs as two half-speed passes.
MMDT = mybir.dt.float32r

_PROG_CACHE: dict = {}


def build_program(cap: int):
    """One SPMD program, run on all 8 cores; per-core data selects the expert.

    Per-core inputs:
      xgT  [512, cap]   gathered tokens for this expert, transposed (d-major)
      w1t  [512, 2048]  w1_e.T
      w2t  [2048, 512]  w2_e.T
      b1r  [16, 128, 1] b1_e
      b2r  [1, 512]     b2_e
      wgt  [cap//128, 128, 1] combine weights per gathered slot (0 for pads)
    Output:
      out  [cap, 512]   scaled expert contributions, row s = token slot s
    """
    nc = bacc.Bacc("TRN2", target_bir_lowering=False, debug=True)

    xgT = nc.dram_tensor("xgT", [D_MODEL, cap], MMDT, kind="ExternalInput")
    w1t = nc.dram_tensor("w1t", [D_MODEL, DFF], MMDT, kind="ExternalInput")
    w2t = nc.dram_tensor("w2t", [DFF, D_MODEL], MMDT, kind="ExternalInput")
    b1r = nc.dram_tensor("b1r", [DFF // 128, 128, 1], FP, kind="ExternalInput")
    b2r = nc.dram_tensor("b2r", [1, D_MODEL], MMDT, kind="ExternalInput")
    wgt = nc.dram_tensor("wgt", [cap // 128, 128, 1], FP, kind="ExternalInput")
    onesr = nc.dram_tensor("onesr", [1, 128], MMDT, kind="ExternalInput")
    out = nc.dram_tensor("out", [cap, D_MODEL], FP, kind="ExternalOutput")

    KD = D_MODEL // 128  # 4 k-slices for mm1
    MD = DFF // 128  # 16 dff tiles
    n_blocks = (cap + 511) // 512  # token blocks of <=512 for mm1 N-dim

    with tile.TileContext(nc) as tc:
        with (
            tc.tile_pool(name="weights", bufs=1) as wpool,
            tc.tile_pool(name="acts", bufs=2) as apool,
            tc.tile_pool(name="h", bufs=2) as hpool,
            tc.tile_pool(name="psum", bufs=4, space="PSUM") as ppool,
            tc.tile_pool(name="outp", bufs=3) as opool,
            tc.tile_pool(name="consts", bufs=1) as cpool,
        ):
            # --- load inputs into SBUF ---
            # Order matters for the PE cold-start: mm1's operands (xgT, w1t)
            # first so matmuls start while w2t (only needed ~30us later)
            # still streams in. Spread across sync+gpsimd DMA paths.
            xgT_sb = []
            for k in range(KD):
                t = wpool.tile([128, cap], MMDT, tag=f"xgT{k}")
                nc.sync.dma_start(out=t[:], in_=xgT[k * 128 : (k + 1) * 128, :])
                xgT_sb.append(t)
            w1t_sb = []
            for k in range(KD):
                t = wpool.tile([128, DFF], MMDT, tag=f"w1t{k}")
                # split each 1MB tile into halves so the first m-tiles of
                # block 0 unblock as early as possible
                nc.sync.dma_start(
                    out=t[:, : DFF // 2], in_=w1t[k * 128 : (k + 1) * 128, : DFF // 2]
                )
                nc.sync.dma_start(
                    out=t[:, DFF // 2 :], in_=w1t[k * 128 : (k + 1) * 128, DFF // 2 :]
                )
                w1t_sb.append(t)
            b1_sb = wpool.tile([128, MD], FP, tag="b1")
            # b1r is [16,128,1]; lay tiles side by side: column m = tile m
            nc.gpsimd.dma_start(
                out=b1_sb[:], in_=b1r.rearrange("m p o -> p (m o)")
            )
            b2_sb = cpool.tile([1, D_MODEL], MMDT, tag="b2")
            nc.gpsimd.dma_start(out=b2_sb[:], in_=b2r[:, :])
            wgt_sb = wpool.tile([128, cap // 128], FP, tag="wgt")
            nc.gpsimd.dma_start(out=wgt_sb[:], in_=wgt.rearrange("u p o -> p (u o)"))
            ones_sb = cpool.tile([1, 128], MMDT, tag="ones")
            nc.gpsimd.dma_start(out=ones_sb[:], in_=onesr[:, :])
            w2t_sb = []
            for m in range(MD):
                t = wpool.tile([128, D_MODEL], MMDT, tag=f"w2t{m}")
                nc.sync.dma_start(out=t[:], in_=w2t[m * 128 : (m + 1) * 128, :])
                w2t_sb.append(t)

            # --- main loop over token blocks ---
            for blk in range(n_blocks):
                ncols = min(512, cap - blk * 512)
                # mm1: h_T[dff, tokens-in-block] = relu(w1t.T @ xgT + b1)
                h_sb = []
                for m in range(MD):
                    ps = ppool.tile([128, ncols], FP, tag="ps1")
                    for k in range(KD):
                        nc.tensor.matmul(
                            ps[:],
                            w1t_sb[k][:, m * 128 : (m + 1) * 128],
                            xgT_sb[k][:, blk * 512 : blk * 512 + ncols],
                            start=(k == 0),
                            stop=(k == KD - 1),
                        )
                    h = hpool.tile([128, ncols], MMDT, tag=f"h{m}")
                    nc.scalar.activation(
                        h[:],
                        ps[:],
                        mybir.ActivationFunctionType.Relu,
                        bias=b1_sb[:, m : m + 1],
                    )
                    h_sb.append(h)
                # mm2: out[tok, d] = (h_T.T @ w2t) + b2, then scale rows
                for t in range(ncols // 128):
                    ps2 = ppool.tile([128, D_MODEL], FP, tag="ps2")
                    for m in range(MD):
                        nc.tensor.matmul(
                            ps2[:],
                            h_sb[m][:, t * 128 : (t + 1) * 128],
                            w2t_sb[m][:],
                            start=(m == 0),
                            stop=False,
                        )
                    # rank-1 bias: ones[1,128].T @ b2[1,512]
                    nc.tensor.matmul(
                        ps2[:],
                        ones_sb[:],
                        b2_sb[:],
                        start=False,
                        stop=True,
                    )
                    o = opool.tile([128, D_MODEL], FP, tag="o")
                    u = blk * 4 + t  # token subtile index
                    nc.scalar.mul(o[:], ps2[:], mul=wgt_sb[:, u : u + 1])
                    nc.sync.dma_start(
                        out=out[u * 128 : (u + 1) * 128, :], in_=o[:]
                    )
    nc.compile()
    return nc


def _route(x_flat: np.ndarray, router_w: np.ndarray):
    """Host-side replica of the reference router: top-2 + renormalized weights."""
    logits = x_flat @ router_w.T  # [L, E]
    m = logits.max(axis=-1, keepdims=True)
    p = np.exp(logits - m)
    p /= p.sum(axis=-1, keepdims=True)
    order = np.argsort(-p, axis=-1)[:, :K]  # [L, K]
    pv = np.take_along_axis(p, order, axis=-1)
    pv = pv / (pv.sum(axis=-1, keepdims=True) + 1e-9)
    return order, pv


def _build_in_maps(x, router_w, w1_all, b1_all, w2_all, b2_all):
    """Shared staging: router + expert-parallel dispatch lists + per-core
    input maps. Returns (cap, in_maps, idx_lists)."""
    x_flat = np.asarray(x, np.float32).reshape(-1, D_MODEL)
    order, pv = _route(x_flat, np.asarray(router_w, np.float32))
    idx_lists, wgt_lists = [], []
    for e in range(E):
        sel = np.nonzero(order == e)
        idx_lists.append(sel[0])
        wgt_lists.append(pv[sel])
    max_n = max(len(t) for t in idx_lists)
    cap = max(CAP, -(-max_n // 128) * 128)
    in_maps = []
    for e in range(E):
        toks, ws = idx_lists[e], wgt_lists[e]
        n_e = len(toks)
        xg = np.zeros((cap, D_MODEL), np.float32)
        xg[:n_e] = x_flat[toks]
        wg = np.zeros((cap,), np.float32)
        wg[:n_e] = ws
        in_maps.append(
            {
                "xgT": np.ascontiguousarray(xg.T),
                "w1t": np.ascontiguousarray(np.asarray(w1_all, np.float32)[e].T),
                "w2t": np.ascontiguousarray(np.asarray(w2_all, np.float32)[e].T),
                "b1r": np.ascontiguousarray(
                    np.asarray(b1_all, np.float32)[e].reshape(DFF // 128, 128, 1)
                ),
                "b2r": np.asarray(b2_all, np.float32)[e].reshape(1, D_MODEL),
                "wgt": wg.reshape(cap // 128, 128, 1),
                "onesr": np.ones((1, 128), np.float32),
            }
        )
    return cap, in_maps, idx_lists


def _get_program(cap: int):
    if cap not in _PROG_CACHE:
        _PROG_CACHE[cap] = build_program(cap)
    return _PROG_CACHE[cap]


def kernel(x, router_w, w1_all, b1_all, w2_all, b2_all):
    from concourse.bass_utils import run_bass_kernel_spmd

    x = np.asarray(x, dtype=np.float32)
    Bb, Nn, C = x.shape

    cap, in_maps, idx_lists = _build_in_maps(
        x, router_w, w1_all, b1_all, w2_all, b2_all
    )
    nc = _get_program(cap)

    res = run_bass_kernel_spmd(nc, in_maps, core_ids=list(range(N_CORES)))

    # Unshard: weighted all-to-all return == scatter-add contributions per token.
    final = np.zeros((Bb * Nn, C), np.float32)
    for e in range(E):
        toks = idx_lists[e]
        final[toks] += res.results[e]["out"][: len(toks)]
    return final.reshape(Bb, Nn, C)


def time_kernel(x, router_w, w1_all, b1_all, w2_all, b2_all, iters: int = 50):
    """Wall-clock the NEFF execution: jit once, device-put inputs, run a
    pipelined loop. Returns estimated ns per execution (all 8 cores)."""
    import time as _time

    import jax
    from jax.experimental.shard_map import shard_map
    from jax.sharding import Mesh, NamedSharding, PartitionSpec

    from concourse import bass2jax

    cap, in_maps, _ = _build_in_maps(x, router_w, w1_all, b1_all, w2_all, b2_all)
    nc = _get_program(cap)

    bass2jax.install_neuronx_cc_hook()

    import concourse.mybir as _mb

    partition_name = nc.partition_id_tensor.name if nc.partition_id_tensor else None
    in_names, out_names, out_avals, zero_outs = [], [], [], []
    for alloc in nc.m.functions[0].allocations:
        if not isinstance(alloc, _mb.MemoryLocationSet):
            continue
        name = alloc.memorylocations[0].name
        if alloc.kind == "ExternalInput":
            if name != partition_name:
                in_names.append(name)
        elif alloc.kind == "ExternalOutput":
            shape = tuple(alloc.tensor_shape)
            dtype = _mb.dt.np(alloc.dtype)
            out_names.append(name)
            out_avals.append(jax.core.ShapedArray(shape, dtype))
            zero_outs.append(np.zeros(shape, dtype))
    n_params = len(in_names)
    all_in_names = list(in_names) + list(out_names)
    if partition_name is not None:
        all_in_names.append(partition_name)
    if nc.dbg_addr is not None:
        extra_dbg = {nc.dbg_addr.name: np.zeros((1, 2), np.uint32)}
        in_maps = [{**m, **extra_dbg} for m in in_maps]

    def _body(*args):
        operands = list(args)
        if partition_name is not None:
            operands.append(bass2jax.partition_id_tensor())
        outs = bass2jax._bass_exec_p.bind(
            *operands,
            out_avals=tuple(out_avals),
            in_names=tuple(all_in_names),
            out_names=tuple(out_names),
            lowering_input_output_aliases=(),
            sim_require_finite=True,
            sim_require_nnan=True,
            nc=nc,
        )
        return tuple(outs)

    devices = jax.devices()[:N_CORES]
    mesh = Mesh(np.asarray(devices), ("core",))
    spec = PartitionSpec("core")
    in_specs = (spec,) * (n_params + len(out_names))
    out_specs = (spec,) * len(out_names)
    fn = jax.jit(
        shard_map(_body, mesh=mesh, in_specs=in_specs, out_specs=out_specs,
                  check_rep=False),
        keep_unused=True,
    )
    sharding = NamedSharding(mesh, spec)
    concat_in = [
        jax.device_put(
            np.concatenate([np.asarray(in_maps[c][n]) for c in range(N_CORES)], axis=0),
            sharding,
        )
        for n in in_names[:n_params]
    ]
    concat_zeros = [
        jax.device_put(
            np.zeros((N_CORES * z.shape[0], *z.shape[1:]), z.dtype), sharding
        )
        for z in zero_outs
    ]
    # warmup + compile
    outs = fn(*concat_in, *concat_zeros)
    jax.block_until_ready(outs)

    t0 = _time.perf_counter()
    for _ in range(iters):
        outs = fn(*concat_in, *concat_zeros)
    jax.block_until_ready(outs)
    dt = _time.perf_counter() - t0
    return dt / iters * 1e9


# revision 12
# speedup vs baseline: 27.0139x; 1.0614x over previous
"""MoE layer (E=8 experts, top-2 routing) on 8 Trainium2 NeuronCores.

Expert-parallel sharding: core e holds expert e's weights (w1/w2/b1/b2).
Tokens are dispatched to the cores of their top-2 experts, each core runs
its expert's FFN on its gathered tokens and scales rows by the combine
weight, and the scaled contributions are summed back per token (the
all-to-all "return") to form the full output.

Shapes (hardcoded per the problem spec):
  x [2, 2048, 512] f32, router_w [8, 512], w1_all [8, 2048, 512],
  b1_all [8, 2048], w2_all [8, 512, 2048], b2_all [8, 512].
"""

import sys

sys.path.insert(0, "/opt/trn_rl_repo")

import numpy as np

import concourse.bass as bass
import concourse.mybir as mybir
import concourse.tile as tile
from concourse import bacc

D_MODEL = 512
DFF = 2048
E = 8
K = 2
L = 2 * 2048  # total tokens
N_CORES = 8

FP = mybir.dt.float32

# Per-expert token capacity (padded). Expected load is L*K/E = 1024 with
# std ~30 under the near-uniform router; seed-0 max count is 1092. The
# program is built for the actual max count rounded up, so this is only
# the floor.
CAP = 1152

# float32r: single-pass fp32 matmul (full rate for moving dim >= 256),
# vs plain fp32 which runs as two half-speed passes.
MMDT = mybir.dt.float32r

_PROG_CACHE: dict = {}


def build_program(cap: int):
    """One SPMD program, run on all 8 cores; per-core data selects the expert.

    Per-core inputs:
      xgT  [512, cap]   gathered tokens for this expert, transposed (d-major)
      w1t  [512, 2048]  w1_e.T
      w2t  [2048, 512]  w2_e.T
      b1r  [16, 128, 1] b1_e
      b2r  [1, 512]     b2_e
      wgt  [cap//128, 128, 1] combine weights per gathered slot (0 for pads)
    Output:
      out  [cap, 512]   scaled expert contributions, row s = token slot s
    """
    nc = bacc.Bacc("TRN2", target_bir_lowering=False, debug=True)

    xgT = nc.dram_tensor("xgT", [D_MODEL, cap], MMDT, kind="ExternalInput")
    w1t = nc.dram_tensor("w1t", [D_MODEL, DFF], MMDT, kind="ExternalInput")
    w2t = nc.dram_tensor("w2t", [DFF, D_MODEL], MMDT, kind="ExternalInput")
    b1r = nc.dram_tensor("b1r", [DFF // 128, 128, 1], FP, kind="ExternalInput")
    b2r = nc.dram_tensor("b2r", [1, D_MODEL], MMDT, kind="ExternalInput")
    wgt = nc.dram_tensor("wgt", [cap // 128, 128, 1], FP, kind="ExternalInput")
    onesr = nc.dram_tensor("onesr", [1, 128], MMDT, kind="ExternalInput")
    out = nc.dram_tensor("out", [cap, D_MODEL], FP, kind="ExternalOutput")

    KD = D_MODEL // 128  # 4 k-slices for mm1
    MD = DFF // 128  # 16 dff tiles
    n_blocks = (cap + 511) // 512  # token blocks of <=512 for mm1 N-dim

    with tile.TileContext(nc) as tc:
        with (
            tc.tile_pool(name="weights", bufs=1) as wpool,
            tc.tile_pool(name="acts", bufs=2) as apool,
            tc.tile_pool(name="h", bufs=2) as hpool,
            tc.tile_pool(name="psum", bufs=4, space="PSUM") as ppool,
            tc.tile_pool(name="outp", bufs=3) as opool,
            tc.tile_pool(name="consts", bufs=1) as cpool,
        ):
            # --- load inputs into SBUF ---
            # Order matters for the PE cold-start: mm1's operands (xgT, w1t)
            # first so matmuls start while w2t (only needed ~30us later)
            # still streams in. Spread across sync+gpsimd DMA paths.
            xgT_sb = [
                wpool.tile([128, cap], MMDT, tag=f"xgT{k}", name=f"xgT_sb{k}")
                for k in range(KD)
            ]
            w1t_sb = [
                wpool.tile([128, DFF], MMDT, tag=f"w1t{k}", name=f"w1t_sb{k}")
                for k in range(KD)
            ]
            # Stream in mm1's operands in block-0-first order: the first
            # PSUM group (m=0, blk=0) needs xgT[k][:, :512] and
            # w1t[k][:, :128] for all k, i.e. ~1.3MB, not the full 6.5MB.
            for k in range(KD):
                nc.sync.dma_start(
                    out=xgT_sb[k][:, :512], in_=xgT[k * 128 : (k + 1) * 128, :512]
                )
            for mc in range(4):  # dff chunks of 512
                for k in range(KD):
                    nc.sync.dma_start(
                        out=w1t_sb[k][:, mc * 512 : (mc + 1) * 512],
                        in_=w1t[k * 128 : (k + 1) * 128, mc * 512 : (mc + 1) * 512],
                    )
            for blk in range(1, (cap + 511) // 512):
                hi = min(cap, blk * 512 + 512)
                for k in range(KD):
                    nc.sync.dma_start(
                        out=xgT_sb[k][:, blk * 512 : hi],
                        in_=xgT[k * 128 : (k + 1) * 128, blk * 512 : hi],
                    )
            b1_sb = wpool.tile([128, MD], FP, tag="b1")
            # b1r is [16,128,1]; lay tiles side by side: column m = tile m
            nc.gpsimd.dma_start(
                out=b1_sb[:], in_=b1r.rearrange("m p o -> p (m o)")
            )
            b2_sb = cpool.tile([1, D_MODEL], MMDT, tag="b2")
            nc.gpsimd.dma_start(out=b2_sb[:], in_=b2r[:, :])
            wgt_sb = wpool.tile([128, cap // 128], FP, tag="wgt")
            nc.gpsimd.dma_start(out=wgt_sb[:], in_=wgt.rearrange("u p o -> p (u o)"))
            ones_sb = cpool.tile([1, 128], MMDT, tag="ones")
            nc.gpsimd.dma_start(out=ones_sb[:], in_=onesr[:, :])
            w2t_sb = []
            for m in range(MD):
                t = wpool.tile([128, D_MODEL], MMDT, tag=f"w2t{m}")
                nc.sync.dma_start(out=t[:], in_=w2t[m * 128 : (m + 1) * 128, :])
                w2t_sb.append(t)

            # --- main loop over token blocks ---
            for blk in range(n_blocks):
                ncols = min(512, cap - blk * 512)
                # mm1: h_T[dff, tokens-in-block] = relu(w1t.T @ xgT + b1)
                h_sb = []
                for m in range(MD):
                    ps = ppool.tile([128, ncols], FP, tag="ps1")
                    for k in range(KD):
                        nc.tensor.matmul(
                            ps[:],
                            w1t_sb[k][:, m * 128 : (m + 1) * 128],
                            xgT_sb[k][:, blk * 512 : blk * 512 + ncols],
                            start=(k == 0),
                            stop=(k == KD - 1),
                        )
                    h = hpool.tile([128, ncols], MMDT, tag=f"h{m}")
                    nc.scalar.activation(
                        h[:],
                        ps[:],
                        mybir.ActivationFunctionType.Relu,
                        bias=b1_sb[:, m : m + 1],
                    )
                    h_sb.append(h)
                # mm2: out[tok, d] = (h_T.T @ w2t) + b2, then scale rows
                for t in range(ncols // 128):
                    ps2 = ppool.tile([128, D_MODEL], FP, tag="ps2")
                    for m in range(MD):
                        nc.tensor.matmul(
                            ps2[:],
                            h_sb[m][:, t * 128 : (t + 1) * 128],
                            w2t_sb[m][:],
                            start=(m == 0),
                            stop=False,
                        )
                    # rank-1 bias: ones[1,128].T @ b2[1,512]
                    nc.tensor.matmul(
                        ps2[:],
                        ones_sb[:],
                        b2_sb[:],
                        start=False,
                        stop=True,
                    )
                    o = opool.tile([128, D_MODEL], FP, tag="o")
                    u = blk * 4 + t  # token subtile index
                    nc.scalar.mul(o[:], ps2[:], mul=wgt_sb[:, u : u + 1])
                    nc.sync.dma_start(
                        out=out[u * 128 : (u + 1) * 128, :], in_=o[:]
                    )
    nc.compile()
    return nc


def _route(x_flat: np.ndarray, router_w: np.ndarray):
    """Host-side replica of the reference router: top-2 + renormalized weights."""
    logits = x_flat @ router_w.T  # [L, E]
    m = logits.max(axis=-1, keepdims=True)
    p = np.exp(logits - m)
    p /= p.sum(axis=-1, keepdims=True)
    order = np.argsort(-p, axis=-1)[:, :K]  # [L, K]
    pv = np.take_along_axis(p, order, axis=-1)
    pv = pv / (pv.sum(axis=-1, keepdims=True) + 1e-9)
    return order, pv


def _build_in_maps(x, router_w, w1_all, b1_all, w2_all, b2_all):
    """Shared staging: router + expert-parallel dispatch lists + per-core
    input maps. Returns (cap, in_maps, idx_lists)."""
    x_flat = np.asarray(x, np.float32).reshape(-1, D_MODEL)
    order, pv = _route(x_flat, np.asarray(router_w, np.float32))
    idx_lists, wgt_lists = [], []
    for e in range(E):
        sel = np.nonzero(order == e)
        idx_lists.append(sel[0])
        wgt_lists.append(pv[sel])
    max_n = max(len(t) for t in idx_lists)
    cap = max(CAP, -(-max_n // 128) * 128)
    in_maps = []
    for e in range(E):
        toks, ws = idx_lists[e], wgt_lists[e]
        n_e = len(toks)
        xg = np.zeros((cap, D_MODEL), np.float32)
        xg[:n_e] = x_flat[toks]
        wg = np.zeros((cap,), np.float32)
        wg[:n_e] = ws
        in_maps.append(
            {
                "xgT": np.ascontiguousarray(xg.T),
                "w1t": np.ascontiguousarray(np.asarray(w1_all, np.float32)[e].T),
                "w2t": np.ascontiguousarray(np.asarray(w2_all, np.float32)[e].T),
                "b1r": np.ascontiguousarray(
                    np.asarray(b1_all, np.float32)[e].reshape(DFF // 128, 128, 1)
                ),
                "b2r": np.asarray(b2_all, np.float32)[e].reshape(1, D_MODEL),
                "wgt": wg.reshape(cap // 128, 128, 1),
                "onesr": np.ones((1, 128), np.float32),
            }
        )
    return cap, in_maps, idx_lists


def _get_program(cap: int):
    if cap not in _PROG_CACHE:
        _PROG_CACHE[cap] = build_program(cap)
    return _PROG_CACHE[cap]


def kernel(x, router_w, w1_all, b1_all, w2_all, b2_all):
    from concourse.bass_utils import run_bass_kernel_spmd

    x = np.asarray(x, dtype=np.float32)
    Bb, Nn, C = x.shape

    cap, in_maps, idx_lists = _build_in_maps(
        x, router_w, w1_all, b1_all, w2_all, b2_all
    )
    nc = _get_program(cap)

    res = run_bass_kernel_spmd(nc, in_maps, core_ids=list(range(N_CORES)))

    # Unshard: weighted all-to-all return == scatter-add contributions per token.
    final = np.zeros((Bb * Nn, C), np.float32)
    for e in range(E):
        toks = idx_lists[e]
        final[toks] += res.results[e]["out"][: len(toks)]
    return final.reshape(Bb, Nn, C)


def time_kernel(x, router_w, w1_all, b1_all, w2_all, b2_all, iters: int = 50):
    """Wall-clock the NEFF execution: jit once, device-put inputs, run a
    pipelined loop. Returns estimated ns per execution (all 8 cores)."""
    import time as _time

    import jax
    from jax.experimental.shard_map import shard_map
    from jax.sharding import Mesh, NamedSharding, PartitionSpec

    from concourse import bass2jax

    cap, in_maps, _ = _build_in_maps(x, router_w, w1_all, b1_all, w2_all, b2_all)
    nc = _get_program(cap)

    bass2jax.install_neuronx_cc_hook()

    import concourse.mybir as _mb

    partition_name = nc.partition_id_tensor.name if nc.partition_id_tensor else None
    in_names, out_names, out_avals, zero_outs = [], [], [], []
    for alloc in nc.m.functions[0].allocations:
        if not isinstance(alloc, _mb.MemoryLocationSet):
            continue
        name = alloc.memorylocations[0].name
        if alloc.kind == "ExternalInput":
            if name != partition_name:
                in_names.append(name)
        elif alloc.kind == "ExternalOutput":
            shape = tuple(alloc.tensor_shape)
            dtype = _mb.dt.np(alloc.dtype)
            out_names.append(name)
            out_avals.append(jax.core.ShapedArray(shape, dtype))
            zero_outs.append(np.zeros(shape, dtype))
    n_params = len(in_names)
    all_in_names = list(in_names) + list(out_names)
    if partition_name is not None:
        all_in_names.append(partition_name)
    if nc.dbg_addr is not None:
        extra_dbg = {nc.dbg_addr.name: np.zeros((1, 2), np.uint32)}
        in_maps = [{**m, **extra_dbg} for m in in_maps]

    def _body(*args):
        operands = list(args)
        if partition_name is not None:
            operands.append(bass2jax.partition_id_tensor())
        outs = bass2jax._bass_exec_p.bind(
            *operands,
            out_avals=tuple(out_avals),
            in_names=tuple(all_in_names),
            out_names=tuple(out_names),
            lowering_input_output_aliases=(),
            sim_require_finite=True,
            sim_require_nnan=True,
            nc=nc,
        )
        return tuple(outs)

    devices = jax.devices()[:N_CORES]
    mesh = Mesh(np.asarray(devices), ("core",))
    spec = PartitionSpec("core")
    in_specs = (spec,) * (n_params + len(out_names))
    out_specs = (spec,) * len(out_names)
    fn = jax.jit(
        shard_map(_body, mesh=mesh, in_specs=in_specs, out_specs=out_specs,
                  check_rep=False),
        keep_unused=True,
    )
    sharding = NamedSharding(mesh, spec)
    concat_in = [
        jax.device_put(
            np.concatenate([np.asarray(in_maps[c][n]) for c in range(N_CORES)], axis=0),
            sharding,
        )
        for n in in_names[:n_params]
    ]
    concat_zeros = [
        jax.device_put(
            np.zeros((N_CORES * z.shape[0], *z.shape[1:]), z.dtype), sharding
        )
        for z in zero_outs
    ]
    # warmup + compile
    outs = fn(*concat_in, *concat_zeros)
    jax.block_until_ready(outs)

    t0 = _time.perf_counter()
    for _ in range(iters):
        outs = fn(*concat_in, *concat_zeros)
    jax.block_until_ready(outs)
    dt = _time.perf_counter() - t0
    return dt / iters * 1e9
